# revision 3
# baseline (speedup 1.0000x reference)
"""Trainium2 Bass kernel for CustomLoss:
    out = mean_{b,t} CE(logits[b,t,:], tgt[b,t]) + penalty
    CE   = logsumexp_V(logits) - logits[tgt]
    penalty = sum_b C(n_b, 2), n_b = #{t : sizes[b, argmax_V logits[b,t,:]] > 0}

Sharding: data-parallel over the 4096 (b,t) tokens -> 512 tokens/core on 8
NeuronCores. Each core streams its [512, 32000] logits shard through SBUF
once at HBM rate (~420 GB/s); DVE computes per-block maxes, ACT computes exp
with fused free-axis accumulation (logsumexp), GPSIMD does the small indirect
gathers. The per-tile argmax uses max/max_index to pick the winning 250-wide
block, then gathers that logits block and the matching sizes block in
parallel and selects the predicted token's size with a one-hot, so only one
round of gather latency sits in the tail. The final tile's last vocab chunk
is only 1000 columns so the post-stream reduce is short. Per-core partial
sums leave as a [128, 2] tile; partitions are summed on host.
"""

from contextlib import ExitStack

import numpy as np

P = 128
V = 32000
B, T = 2, 2048
N_CORES = 8
TOK = (B * T) // N_CORES      # 512 tokens per core
NT = TOK // P                 # 4 token tiles of 128 partitions
W = 250                       # argmax block width
NB = V // W                   # 128 blocks per token row
VC = 8000                     # default vocab chunk per DMA/compute step
# per-tile (vocab_offset, width) chunk lists; the last tile ends with a tiny
# chunk so the tail reduce after the DMA stream drains is short
_FULL = [(0, 8000), (8000, 8000), (16000, 8000), (24000, 8000)]
_LAST = [(0, 8000), (8000, 8000), (16000, 8000), (24000, 7000), (31000, 1000)]
CHUNKS = [_FULL, _FULL, _FULL, _LAST]
MAXCH = max(len(c) for c in CHUNKS)
ALPHA = 1.0

_NC_CACHE = {}


def _build_nc():
    """Build the single-core Bass program (identical on all 8 cores)."""
    import concourse.bacc as bacc
    import concourse.bass as bass
    import concourse.mybir as mybir
    import concourse.tile as tile

    f32 = mybir.dt.float32
    i32 = mybir.dt.int32
    u32 = mybir.dt.uint32
    AF = mybir.ActivationFunctionType
    ALU = mybir.AluOpType
    AX = mybir.AxisListType

    nc = bacc.Bacc("TRN2", target_bir_lowering=False)
    logits = nc.declare_dram_parameter("logits", [TOK, V], f32, isOutput=False)
    # flat element index t*V + tgt[t], laid out [p, tile] (token = tt*128 + p)
    tgt_off = nc.declare_dram_parameter("tgt_off", [P, NT], i32, isOutput=False)
    sizes_r = nc.declare_dram_parameter("sizes_r", [NB, W], f32, isOutput=False)
    out = nc.declare_dram_parameter("out", [P, 2], f32, isOutput=True)

    with tile.TileContext(nc) as tc, ExitStack() as ctx:
        lp = ctx.enter_context(tc.tile_pool(name="lp", bufs=4))
        ep = ctx.enter_context(tc.tile_pool(name="ep", bufs=1))
        sm = ctx.enter_context(tc.tile_pool(name="sm", bufs=4))
        cst = ctx.enter_context(tc.tile_pool(name="cst", bufs=1))

        # ---- persistent accumulators / constants (off the DMA stream) ----
        iota_w_i = cst.tile([P, W], i32)
        nc.gpsimd.iota(iota_w_i[:], pattern=[[1, W]], base=0, channel_multiplier=0)
        iota_wf = cst.tile([P, W], f32)
        nc.vector.tensor_copy(iota_wf[:], iota_w_i[:])
        # row base (flat element index) for each (partition, tile):
        # rb[p, tt] = (tt*P + p) * V  -- exact in f32 (max < 2^24).
        # iota free-axis steps are int16-limited, so one iota per tile column.
        rb_i = cst.tile([P, NT], i32)
        for tt in range(NT):
            nc.gpsimd.iota(
                rb_i[:, tt : tt + 1], pattern=[[1, 1]], base=tt * P * V,
                channel_multiplier=V,
            )
        rb_f = cst.tile([P, NT], f32)
        nc.vector.tensor_copy(rb_f[:], rb_i[:])

        tgt_idx = cst.tile([P, NT], i32)
        # scalar (ACT) HWDGE queue, so the sync queue's first job is chunk 0
        nc.scalar.dma_start(tgt_idx[:], tgt_off[:, :])
        tgt_logit = cst.tile([P, NT], f32)
        for tt in range(NT):
            nc.gpsimd.indirect_dma_start(
                out=tgt_logit[:, tt : tt + 1],
                out_offset=None,
                in_=logits[:, :],
                in_offset=bass.IndirectOffsetOnAxis(
                    ap=tgt_idx[:, tt : tt + 1], axis=1
                ),
            )

        tot_cols = cst.tile([P, NT], f32)   # per-tile sum(exp) totals
        m_cols = cst.tile([P, NT], f32)     # per-tile positive-size indicator
        acc = cst.tile([P, 2], f32)

        for tt in range(NT):
            chunks = CHUNKS[tt]
            bmax = sm.tile([P, NB], f32, tag="bmax")
            sexp = sm.tile([P, MAXCH], f32, tag="sexp")
            for c, (off, vc) in enumerate(chunks):
                lt = lp.tile([P, VC], f32, tag="lt")
                nc.sync.dma_start(
                    lt[:, :vc], logits[tt * P : (tt + 1) * P, off : off + vc]
                )
                # per-block max in one pass: [P, nb, W] -> [P, nb]
                lt3 = lt[:, :vc].rearrange("p (b w) -> p b w", w=W)
                nc.vector.tensor_reduce(
                    bmax[:, off // W : (off + vc) // W], lt3, axis=AX.X, op=ALU.max
                )
                et = ep.tile([P, VC], f32, tag="et")
                nc.scalar.activation(
                    et[:, :vc], lt[:, :vc], AF.Exp, accum_out=sexp[:, c : c + 1]
                )

            nc.vector.reduce_sum(
                tot_cols[:, tt : tt + 1], sexp[:, : len(chunks)], axis=AX.X
            )

            # ---- argmax: top-8 over block maxes -> winning block id ----
            top8 = sm.tile([P, 8], f32, tag="top8")
            nc.vector.max(top8[:], bmax[:])
            bix8 = sm.tile([P, 8], u32, tag="bix8")
            nc.vector.max_index(bix8[:], top8[:], bmax[:])
            # sizes-block gather can go as soon as the block id is int32
            bid_i = sm.tile([P, 1], i32, tag="bid_i")
            nc.vector.tensor_copy(bid_i[:], bix8[:, 0:1])
            szb = sm.tile([P, W], f32, tag="szb")
            nc.gpsimd.indirect_dma_start(
                out=szb[:],
                out_offset=None,
                in_=sizes_r[:, :],
                in_offset=bass.IndirectOffsetOnAxis(ap=bid_i[:, 0:1], axis=0),
            )
            # logits-block gather: flat elem idx = rb + bid*W, exact in f32
            bidf = sm.tile([P, 1], f32, tag="bidf")
            nc.vector.tensor_copy(bidf[:], bix8[:, 0:1])
            gsf = sm.tile([P, 1], f32, tag="gsf")
            nc.vector.tensor_scalar(
                gsf[:], bidf[:], float(W), rb_f[:, tt : tt + 1],
                op0=ALU.mult, op1=ALU.add,
            )
            gsi = sm.tile([P, 1], i32, tag="gsi")
            nc.vector.tensor_copy(gsi[:], gsf[:])
            blk = sm.tile([P, W], f32, tag="blk")
            nc.gpsimd.indirect_dma_start(
                out=blk[:],
                out_offset=None,
                in_=logits[:, :],
                in_offset=bass.IndirectOffsetOnAxis(ap=gsi[:, 0:1], axis=1),
            )
            mb = sm.tile([P, W], f32, tag="mb")
            nc.vector.tensor_scalar(mb[:], szb[:], 0.0, None, op0=ALU.is_gt)

            # local argmax inside the winning block
            blk8 = sm.tile([P, 8], f32, tag="blk8")
            nc.vector.max(blk8[:], blk[:])
            lix8 = sm.tile([P, 8], u32, tag="lix8")
            nc.vector.max_index(lix8[:], blk8[:], blk[:])
            lixf = sm.tile([P, 1], f32, tag="lixf")
            nc.vector.tensor_copy(lixf[:], lix8[:, 0:1])
            # m = (sizes at predicted idx) > 0, selected with a one-hot
            oh = sm.tile([P, W], f32, tag="oh")
            nc.vector.tensor_scalar(
                oh[:], iota_wf[:], lixf[:, 0:1], None, op0=ALU.is_equal
            )
            prod = sm.tile([P, W], f32, tag="prod")
            nc.vector.tensor_tensor(prod[:], oh[:], mb[:], op=ALU.mult)
            nc.vector.tensor_reduce(
                m_cols[:, tt : tt + 1], prod[:], axis=AX.X, op=ALU.add
            )

        # ---- nll, batched: one Ln activation for all tiles ----
        lse_cols = cst.tile([P, NT], f32)
        nc.scalar.activation(lse_cols[:], tot_cols[:], AF.Ln)
        nll_cols = cst.tile([P, NT], f32)
        nc.vector.tensor_tensor(
            nll_cols[:], lse_cols[:], tgt_logit[:], op=ALU.subtract
        )
        nc.vector.reduce_sum(acc[:, 0:1], nll_cols[:], axis=AX.X)
        nc.vector.reduce_sum(acc[:, 1:2], m_cols[:], axis=AX.X)
        nc.sync.dma_start(out[:, :], acc[:])

    nc.finalize()
    return nc


def _get_nc():
    if "nc" not in _NC_CACHE:
        _NC_CACHE["nc"] = _build_nc()
    return _NC_CACHE["nc"]


def _make_in_maps(logits, tgt, sizes):
    logits = np.ascontiguousarray(np.asarray(logits, dtype=np.float32))
    tgt = np.asarray(tgt).astype(np.int64)
    sizes = np.ascontiguousarray(np.asarray(sizes, dtype=np.float32))

    flat_logits = logits.reshape(B * T, V)
    flat_tgt = tgt.reshape(B * T)

    in_maps = []
    for cid in range(N_CORES):
        lo = cid * TOK
        shard = flat_logits[lo : lo + TOK]                       # [TOK, V]
        toff = (np.arange(TOK, dtype=np.int64) * V + flat_tgt[lo : lo + TOK])
        toff = toff.astype(np.int32).reshape(NT, P).T.copy()     # [P, NT]
        b = (lo) // T
        assert (lo + TOK - 1) // T == b, "shard must not straddle batch rows"
        in_maps.append(
            {
                "logits": shard,
                "tgt_off": toff,
                "sizes_r": sizes[b].reshape(NB, W),
            }
        )
    return in_maps


def _combine(results):
    nll_total = 0.0
    counts = np.zeros(B, dtype=np.float64)
    for cid, res in enumerate(results):
        o = np.asarray(res["out"], dtype=np.float64)             # [P, 2]
        nll_total += o[:, 0].sum()
        counts[(cid * TOK) // T] += o[:, 1].sum()
    ce = nll_total / (B * T)
    penalty = float(sum(n * (n - 1) / 2 for n in counts))
    return np.float32(ce + ALPHA * penalty)


def run(logits, tgt, sizes, trace=False):
    """Run the SPMD kernel on 8 cores. Returns (output_scalar, exec_time_ns)."""
    from concourse.bass_utils import run_bass_kernel_spmd

    nc = _get_nc()
    in_maps = _make_in_maps(logits, tgt, sizes)
    r = run_bass_kernel_spmd(nc, in_maps, list(range(N_CORES)), trace=trace)
    _NC_CACHE["last_result"] = r
    return _combine(r.results), r.exec_time_ns


def kernel(logits, tgt, sizes):
    out, _ = run(logits, tgt, sizes, trace=False)
    return out
